# revision 33
# baseline (speedup 1.0000x reference)
"""GQA causal attention (S=2048, H=32, KVH=8, D=128) on 8 TRN2 NeuronCores.

Sharding: tensor-parallel over heads. Core i computes query heads
[4i, 4i+4) against KV head i (GQA group size 32/8 = 4). No collectives:
the host slices the inputs per core and concatenates the outputs.

Per-core algorithm (seq=2048, d=128, 4 q-heads, 1 kv-head, causal):
  - K^T and per-head Q^T staged in SBUF as [d=128, seq] bf16 via PE
    transposes (fp32 DMA-transpose unsupported; XBAR dma transposes
    globally serialize the DMA subsystem -- measured 1.8x SLOWER).
  - Scores S^T are packed TIGHTLY across key-tiles into a contiguous
    per-head column space of 17408 = sum_kt (2048-128*kt) columns.
    Score production walks this space in 1024-col chunks through two
    [128,1024] PSUM tiles (2 banks each); QK matmuls split at PSUM bank
    boundaries (one matmul may not cross a bank).
  - exp() runs on ScalarE as ONE wide ACTIVATE per 1024-col chunk,
    writing a contiguous per-head P^T arena [128, 17408] bf16 in SBUF
    (triple-buffered across heads so a new head's first QK chunk never
    waits for the PV lag of head h-2 to finish reading the recycled
    buffer). The ACT stream is the steady-state gate: ScalarE issues
    back-to-back at ~(N+172)/1.2 ns per chunk (~68us of 101us total).
    Widening chunks would cut the per-ACT overhead but needs PSUM banks
    the transposes/PV occupy (8-bank budget: scores 2x2, PV 2, tp 2).
  - Diagonal 128-col blocks are causal-masked in place by a GpSimd
    affine_select (keeps key<=query, else 0) as soon as their 128 cols
    are exp'd. ONLY masks live on GpSimd: its f32->bf16 casts run
    ~1.9us/chunk (4.4x slower than DVE) and anything slow on that queue
    delays masks, which PV matmuls head-of-line-wait on (stalling the
    PE and therefore the ACT stream). All casts stay on the DVE.
  - PV: for each query-tile qt: acc[qt] = sum_k2 (arena slice).T @ [V|1]
    in a per-qt PSUM accumulator from a 2-buf pool (banks alternate, so
    the PE's next accumulation never shares a bank with the DVE's
    normalization reads); the DVE normalizes straight out of PSUM.
    PV matmuls pipeline at ~60ns each (streaming floor; LDWEIGHTS hides
    under the PE's reorder window). Groups of >= PV_SPLIT key-tiles are
    split into two half-accumulations summed on the DVE so a drain
    never dumps a ~1us matmul burst on the PE at head transitions.
  - Engine-queue discipline (the hard-won part): every engine queue is
    IN-ORDER, so emission order must match data-readiness order. The
    DVE queue orders casts and transpose-copies so nothing waits behind
    an op whose DMA is still in flight; prologue K/V DMAs ride the
    ScalarE hwdge queue (idle until the first ACT) and never queue
    behind Q's on Sync; steady-state DMAs all ride Sync (a DMA dispatch
    on the ScalarE queue would delay the ACT stream). In-flight DMAs
    fair-share HBM, so loads are staggered across chunks rather than
    issued up front (critical-first issue does NOT work -- dispatch is
    immediate and everything fair-shares anyway).
  - Prologue: 8 WIDE (512-col) warmup matmuls cover the DMA wait so the
    HAM clock-gate reaches 2.4 GHz before real QK work; narrow warmups
    or transposes do NOT register as PE-busy for the HAM and it stays
    at 1.2 GHz for ~10us of the stream. First ACT at ~10.5us graded.
  - PV lags score production by 3 key-tiles (1 on the last head, with 2
    drains/chunk); next head's Q prep is spread over chunks 4-15.
"""

import numpy as np

SEQ = 2048
D = 128
QH = 4  # query heads per core
N_CORES = 8
SCALE = 0.08838834764831845  # 1/sqrt(128)
NT = SEQ // 128  # 16 tiles of 128 along seq

SLOT = 512              # fp32 per PSUM bank
CHUNK = 1024            # score chunk: one pool tile (2 banks), one EXP
ARENA = sum(SEQ - 128 * t for t in range(NT))  # 17408 packed score cols/head
NCH = ARENA // CHUNK    # 17 chunks per head

_NC = None


def _emit(ctx, tc, q, k, v, out):
    import concourse.mybir as mybir
    from concourse import masks

    nc = tc.nc
    f32 = mybir.dt.float32
    bf16 = mybir.dt.bfloat16
    Exp = mybir.ActivationFunctionType.Exp

    spans = [SEQ - 128 * t for t in range(NT)]
    offs = [0] * NT
    for t in range(1, NT):
        offs[t] = offs[t - 1] + spans[t - 1]

    singles = ctx.enter_context(tc.tile_pool(name="singles", bufs=1))
    qpool = ctx.enter_context(tc.tile_pool(name="qpool", bufs=2))
    apool = ctx.enter_context(tc.tile_pool(name="apool", bufs=3))
    opool = ctx.enter_context(tc.tile_pool(name="opool", bufs=3))
    # PSUM budget (8 banks): scores 2x2 + PV accumulator 2x1 + transposes 2x1.
    psum_s = ctx.enter_context(tc.tile_pool(name="psum_s", bufs=2, space="PSUM"))
    psum_o = ctx.enter_context(tc.tile_pool(name="psum_o", bufs=2, space="PSUM"))
    psum_t = ctx.enter_context(tc.tile_pool(name="psum_t", bufs=2, space="PSUM"))

    ident = singles.tile([128, 128], bf16)
    masks.make_identity(nc, ident[:])

    # PE warmup: WIDE dummy matmuls covering the DMA prologue so the HAM
    # clock-gate reaches 2.4 GHz by the time transposes + QK arrive.
    # 512-col moving operands keep the PE-busy density high enough to
    # trigger the un-throttle (128-col ones measurably do NOT).
    warm_src = singles.tile([128, 512], bf16, tag="warm_src")

    def warmup(n):
        for i in range(n):
            wsp = psum_s.tile([128, CHUNK], f32, tag="s")
            nc.tensor.matmul(
                wsp[:, 0:512], lhsT=warm_src[:, 0:128], rhs=warm_src[:],
                start=True, stop=True,
            )

    kT = singles.tile([128, SEQ], bf16)
    knat = singles.tile([128, NT, 128], f32, tag="knat")
    knat_bf = singles.tile([128, NT, 128], bf16, tag="knat_bf")
    kr = k.rearrange("(t p) d -> p t d", p=128)

    def kdma(t0, n, eng, cast=None):
        """Load + cast K tiles [t0, t0+n). Prologue loads ride the ScalarE
        hwdge queue (idle until the first ACT); steady-state loads must use
        Sync or they would delay the ACT stream. The cast engine is chosen
        per call: DVE for the latency-critical prologue chain, GpSimd for
        steady-state prefetch (so it never blocks DVE transpose copies)."""
        cs = slice(t0, t0 + n)
        eng.dma_start(out=knat[:, cs, :], in_=kr[:, cs, :])
        (cast or nc.vector).tensor_copy(knat_bf[:, cs, :], knat[:, cs, :])

    def ktp(t0, n):
        """PE-transpose K tiles [t0, t0+n) into kT."""
        for t in range(t0, t0 + n):
            pst = psum_t.tile([128, 128], bf16, tag="tp")
            nc.tensor.transpose(pst[:], knat_bf[:, t, :], ident[:])
            nc.vector.tensor_copy(kT[:, t * 128:(t + 1) * 128], pst[:])

    def qprep_alloc(h):
        qnat = singles.tile([128, NT, 128], f32, tag=f"qnat{h}")
        qnat_bf = singles.tile([128, NT, 128], bf16, tag=f"qnat_bf{h}")
        qT = qpool.tile([128, SEQ], bf16, tag="qT")
        return qnat, qnat_bf, qT

    def qprep_dma(h, st, c, cast=None):
        """Load + cast one 4-tile chunk of head h's Q. Steady-state casts
        go to GpSimd; head-0's latency-critical ones to the DVE."""
        qnat, qnat_bf, qT = st
        qrh = q[:, h * D:(h + 1) * D].rearrange("(t p) d -> p t d", p=128)
        cs = slice(c * 4, (c + 1) * 4)
        nc.sync.dma_start(out=qnat[:, cs, :], in_=qrh[:, cs, :])
        (cast or nc.vector).tensor_copy(qnat_bf[:, cs, :], qnat[:, cs, :])

    def qprep_tp(h, st, t0, n):
        """PE-transpose tiles [t0, t0+n) of head h's Q into qT."""
        qnat, qnat_bf, qT = st
        for t in range(t0, t0 + n):
            pst = psum_t.tile([128, 128], bf16, tag="tp")
            nc.tensor.transpose(pst[:], qnat_bf[:, t, :], ident[:])
            nc.vector.tensor_copy(qT[:, t * 128:(t + 1) * 128], pst[:])

    # ---- V: natural [128, t, d] bf16 + ones column for the denominator
    vp = singles.tile([128, NT, D + 1], bf16)
    vnat = singles.tile([128, NT, 128], f32, tag="vnat")

    def vprep_dma():
        nc.sync.dma_start(out=vnat[:], in_=v.rearrange("(t p) d -> p t d", p=128))
        nc.vector.memset(vp[:, :, D:D + 1], 1.0)

    def vprep_copy():
        # one wide DVE copy, scheduled AFTER the head-0 transpose copies it
        # would otherwise block on the in-order DVE queue. GpSimd is wrong
        # for this: its f32->bf16 casts run ~1.9us per 4-tile piece and
        # delay the causal masks (PV matmuls head-of-line-wait on them).
        nc.vector.tensor_copy(vp[:, :, 0:D], vnat[:])

    # Pending-PV work: a queue of sub-group entries
    # (h, qt, arena, klo, khi, final). Large qt groups (>= PV_SPLIT key
    # tiles) are split into two half-accumulations summed on the DVE, so
    # a single drain never dumps a >0.5us matmul burst on the PE (whole
    # big groups stall the ACT stream at head transitions otherwise).
    PV_SPLIT = 13
    pvq = []
    pv_state = {}

    def pv_queue(h2, qt2, arena2):
        if qt2 >= PV_SPLIT:
            half = (qt2 + 1) // 2
            pvq.append((h2, qt2, arena2, 0, half, False))
            pvq.append((h2, qt2, arena2, half, qt2 + 1, True))
        else:
            pvq.append((h2, qt2, arena2, 0, qt2 + 1, True))

    def pv_advance(budget_mms, max_groups=2):
        """Pop sub-group entries (each an atomic accumulation run) until
        the MM budget is spent. O[qt] = sum_k2 arena[k2-slice].T @ [V|1];
        the ones column gives the softmax denominator for free."""
        left = budget_mms
        groups = 0
        while left > 0 and groups < max_groups and pvq:
            groups += 1
            h2, qt2, arena2, klo, khi, final = pvq.pop(0)
            st = pv_state.setdefault(h2, {})
            ops = psum_o.tile([128, D + 1], f32, tag="o")
            if qt2 % 2 == 0 and klo == 0:
                st["osb"] = opool.tile([128, 2, D], f32, tag="osb", name="osb")
            for kk in range(klo, khi):
                a0 = offs[kk] + (qt2 - kk) * 128
                nc.tensor.matmul(
                    ops[:], lhsT=arena2[:, a0:a0 + 128], rhs=vp[:, kk, :],
                    start=(kk == klo), stop=(kk == khi - 1),
                )
            left -= khi - klo
            if not final:
                # first half: bounce to SBUF so the PSUM bank frees fast
                half_sb = opool.tile([128, D + 1], f32, tag="half", name="half_sb")
                nc.vector.tensor_copy(half_sb[:], ops[:])
                st[("half", qt2)] = half_sb
                continue
            if klo > 0:
                half_sb = st.pop(("half", qt2))
                tot = opool.tile([128, D + 1], f32, tag="tot")
                nc.vector.scalar_tensor_tensor(
                    tot[:], ops[:], 1.0, half_sb[:],
                    mybir.AluOpType.mult, mybir.AluOpType.add,
                )
                src = tot
            else:
                src = ops
            osb = st["osb"]
            rec = opool.tile([128, 1], f32, tag="rec")
            nc.vector.reciprocal(rec[:], src[:, D:D + 1])
            nc.vector.tensor_scalar_mul(osb[:, qt2 % 2, :], src[:, 0:D], rec[:])
            if h2 == QH - 1 and qt2 >= 14:
                # overlap the last stores with the trailing normalizations
                nc.sync.dma_start(
                    out=out[qt2 * 128:(qt2 + 1) * 128, h2 * D:(h2 + 1) * D],
                    in_=osb[:, qt2 % 2, :],
                )
            elif qt2 % 2 == 1:
                qb = qt2 // 2
                nc.sync.dma_start(
                    out=out[qb * 256:(qb + 1) * 256, h2 * D:(h2 + 1) * D].rearrange(
                        "(j p) d -> p j d", p=128
                    ),
                    in_=osb[:],
                )

    # Prologue, critical-bytes first: chunk 0's QK needs ONLY K tile 0
    # (64KB) + Q tiles 0-7; chunk 1 adds Q tiles 8-15. In-flight DMAs
    # fair-share the HBM fabric, so everything else (K tiles 1-15, V) is
    # deferred into the chunk loop rather than issued up front — issuing
    # it all at once starves the critical transfers. K rides the ScalarE
    # hwdge queue (idle until the first ACT), Q rides Sync.
    nc.vector.memset(warm_src[:], 0.0)
    kdma(0, 1, nc.scalar, cast=nc.vector)
    q0st = qprep_alloc(0)
    qprep_dma(0, q0st, 0, cast=nc.vector)
    qprep_dma(0, q0st, 1, cast=nc.vector)
    warmup(8)
    ktp(0, 1)
    qprep_tp(0, q0st, 0, 4)
    qprep_tp(0, q0st, 4, 4)
    # c2/c3 casts are emitted AFTER the tile 0-7 transpose copies so they
    # never block them on the in-order DVE queue; tiles 8-11 transpose in
    # the PE's idle window before chunk 0's QK.
    qprep_dma(0, q0st, 2, cast=nc.vector)
    qprep_dma(0, q0st, 3, cast=nc.vector)
    qprep_tp(0, q0st, 8, 4)
    qT = q0st[2]

    for h in range(QH):
        arena = apool.tile([128, ARENA], bf16, tag="arena")
        qT_next = None
        qst_next = None
        done_kt = 0       # key-tiles fully exp'd so far
        done_mask = 0     # key-tiles whose diagonal block is masked
        for ci in range(NCH):
            c0, c1 = ci * CHUNK, (ci + 1) * CHUNK
            # late prologue interleave (head 0 only): K DMAs go early so the
            # casts/transposes never stall the PE mid-head.
            if h == 0:
                if ci == 0:
                    kdma(1, 3, nc.scalar, cast=nc.vector)
                    vprep_dma()
                elif ci == 1:
                    # chunk 1 reads qT cols 1024-2048: tiles 12-15 must be
                    # transposed before this chunk's QK matmuls; kT tiles
                    # 1-2 before chunks 2-3's.
                    qprep_tp(0, q0st, 12, 4)
                    ktp(1, 2)
                elif ci == 2:
                    vprep_copy()
                elif ci == 3:
                    kdma(4, 4, nc.sync, cast=nc.vector)
                elif ci == 4:
                    ktp(3, 1)
                elif ci == 5:
                    kdma(8, 4, nc.sync, cast=nc.vector)
                    ktp(4, 4)
                elif ci == 9:
                    kdma(12, 4, nc.sync, cast=nc.vector)
                    ktp(8, 4)
                elif ci == 12:
                    ktp(12, 4)
            sp = psum_s.tile([128, CHUNK], f32, tag="s")
            # QK matmul fragments: split at PSUM bank boundaries
            for kt in range(NT):
                s0, s1 = max(c0, offs[kt]), min(c1, offs[kt] + spans[kt])
                if s0 >= s1:
                    continue
                p = s0
                while p < s1:
                    w = min(s1 - p, SLOT - (p % SLOT))
                    qs = kt * 128 + (p - offs[kt])
                    nc.tensor.matmul(
                        sp[:, p - c0:p - c0 + w],
                        lhsT=kT[:, kt * 128:(kt + 1) * 128],
                        rhs=qT[:, qs:qs + w],
                        start=True, stop=True,
                    )
                    p += w
            # one wide exp over the whole chunk
            nc.scalar.activation(arena[:, c0:c1], sp[:], Exp, scale=SCALE)
            # mask diagonal blocks in place (GpSimd, idle queue) as soon as
            # their 128 cols are exp'd, and queue PV on full completion.
            while done_mask < NT and offs[done_mask] + 128 <= c1:
                o0 = offs[done_mask]
                nc.gpsimd.affine_select(
                    out=arena[:, o0:o0 + 128], in_=arena[:, o0:o0 + 128],
                    compare_op=mybir.AluOpType.is_ge, fill=0.0,
                    base=0, channel_multiplier=-1, pattern=[[1, 128]],
                )
                done_mask += 1
            while done_kt < NT and offs[done_kt] + spans[done_kt] <= c1:
                pv_queue(h, done_kt, arena)
                done_kt += 1
            # lag-based PV drain
            lag = 1 if h == QH - 1 else 3
            drains = 2 if h == QH - 1 else 1
            for _ in range(drains):
                if len(pvq) > lag:
                    pv_advance(17, max_groups=1)
            # spread the next head's Q prep over late chunks: DMAs at
            # ci 4/6/8/10 (two chunks of slack before their transposes
            # need the casts), two transposes per chunk at ci 8..15.
            if h + 1 < QH:
                if ci == 4:
                    qst_next = qprep_alloc(h + 1)
                    qT_next = qst_next[2]
                if ci in (4, 6, 8, 10):
                    qprep_dma(h + 1, qst_next, (ci - 4) // 2)
                if 8 <= ci <= 15:
                    qprep_tp(h + 1, qst_next, (ci - 8) * 2, 2)
        if qT_next is not None:
            qT = qT_next
    while pvq:
        pv_advance(17, max_groups=1)


def _build():
    import concourse.mybir as mybir
    import concourse.tile as tile
    from concourse import bacc
    from contextlib import ExitStack

    nc = bacc.Bacc()
    q = nc.declare_dram_parameter("q", [SEQ, QH * D], mybir.dt.float32, isOutput=False)
    k = nc.declare_dram_parameter("k", [SEQ, D], mybir.dt.float32, isOutput=False)
    v = nc.declare_dram_parameter("v", [SEQ, D], mybir.dt.float32, isOutput=False)
    out = nc.declare_dram_parameter("out", [SEQ, QH * D], mybir.dt.float32, isOutput=True)

    with tile.TileContext(nc) as tc:
        with ExitStack() as ctx:
            _emit(ctx, tc, q[:], k[:], v[:], out[:])
    nc.compile()
    return nc


def _get_nc():
    global _NC
    if _NC is None:
        _NC = _build()
    return _NC


def _ensure_ntff_hook():
    """The agent image's antenv lacks axon_hooks; shim it so trace=True works."""
    import sys
    import types

    if "antenv.axon_hooks" in sys.modules:
        return
    try:
        import antenv
        from trn_agent_boot.trn_boot import _ntff_profile_via_ctypes
    except ImportError:
        return
    mod = types.ModuleType("antenv.axon_hooks")
    hook = [None]
    mod.set_axon_ntff_profile_hook = lambda h: hook.__setitem__(0, h)
    mod.get_axon_ntff_profile_hook = lambda: hook[0]
    sys.modules["antenv.axon_hooks"] = mod
    antenv.axon_hooks = mod
    mod.set_axon_ntff_profile_hook(_ntff_profile_via_ctypes("/opt/axon/libaxon_pjrt.so"))


def _run(q, k, v, trace=False):
    from concourse.bass_utils import run_bass_kernel_spmd

    if trace:
        _ensure_ntff_hook()
    nc = _get_nc()
    in_maps = []
    for i in range(N_CORES):
        in_maps.append(
            {
                "q": np.ascontiguousarray(q[:, i * QH * D:(i + 1) * QH * D]).astype(np.float32, copy=False),
                "k": np.ascontiguousarray(k[:, i * D:(i + 1) * D]).astype(np.float32, copy=False),
                "v": np.ascontiguousarray(v[:, i * D:(i + 1) * D]).astype(np.float32, copy=False),
            }
        )
    res = run_bass_kernel_spmd(nc, in_maps, core_ids=list(range(N_CORES)), trace=trace)
    full = np.concatenate([res.results[i]["out"] for i in range(N_CORES)], axis=1)
    return full.astype(np.float32, copy=False), res


def kernel(q, k, v):
    out, _ = _run(q, k, v, trace=False)
    return out


# revision 34
# speedup vs baseline: 1.0098x; 1.0098x over previous
"""GQA causal attention (S=2048, H=32, KVH=8, D=128) on 8 TRN2 NeuronCores.

Sharding: tensor-parallel over heads. Core i computes query heads
[4i, 4i+4) against KV head i (GQA group size 32/8 = 4). No collectives:
the host slices the inputs per core and concatenates the outputs.

Per-core algorithm (seq=2048, d=128, 4 q-heads, 1 kv-head, causal):
  - K^T and per-head Q^T staged in SBUF as [d=128, seq] bf16 via PE
    transposes (fp32 DMA-transpose unsupported; XBAR dma transposes
    globally serialize the DMA subsystem -- measured 1.8x SLOWER).
  - Scores S^T are packed TIGHTLY across key-tiles into a contiguous
    per-head column space of 17408 = sum_kt (2048-128*kt) columns.
    Score production walks this space in 1024-col chunks through two
    [128,1024] PSUM tiles (2 banks each); QK matmuls split at PSUM bank
    boundaries (one matmul may not cross a bank).
  - exp() runs on ScalarE as ONE wide ACTIVATE per 1024-col chunk,
    writing a contiguous per-head P^T arena [128, 17408] bf16 in SBUF
    (triple-buffered across heads so a new head's first QK chunk never
    waits for the PV lag of head h-2 to finish reading the recycled
    buffer). The ACT stream is the steady-state gate: ScalarE issues
    back-to-back at ~(N+172)/1.2 ns per chunk (~68us of 101us total).
    Widening chunks would cut the per-ACT overhead but needs PSUM banks
    the transposes/PV occupy (8-bank budget: scores 2x2, PV 2, tp 2).
  - Diagonal 128-col blocks are causal-masked in place by a GpSimd
    affine_select (keeps key<=query, else 0) as soon as their 128 cols
    are exp'd. ONLY masks live on GpSimd: its f32->bf16 casts run
    ~1.9us/chunk (4.4x slower than DVE) and anything slow on that queue
    delays masks, which PV matmuls head-of-line-wait on (stalling the
    PE and therefore the ACT stream). All casts stay on the DVE.
  - PV: for each query-tile qt: acc[qt] = sum_k2 (arena slice).T @ [V|1]
    in a per-qt PSUM accumulator from a 2-buf pool (banks alternate, so
    the PE's next accumulation never shares a bank with the DVE's
    normalization reads); the DVE normalizes straight out of PSUM.
    PV matmuls pipeline at ~60ns each (streaming floor; LDWEIGHTS hides
    under the PE's reorder window). Groups of >= PV_SPLIT key-tiles are
    split into two half-accumulations summed on the DVE so a drain
    never dumps a ~1us matmul burst on the PE at head transitions.
  - Engine-queue discipline (the hard-won part): every engine queue is
    IN-ORDER, so emission order must match data-readiness order. The
    DVE queue orders casts and transpose-copies so nothing waits behind
    an op whose DMA is still in flight; prologue K/V DMAs ride the
    ScalarE hwdge queue (idle until the first ACT) and never queue
    behind Q's on Sync; steady-state DMAs all ride Sync (a DMA dispatch
    on the ScalarE queue would delay the ACT stream). In-flight DMAs
    fair-share HBM, so loads are staggered across chunks rather than
    issued up front (critical-first issue does NOT work -- dispatch is
    immediate and everything fair-shares anyway).
  - Prologue: 8 WIDE (512-col) warmup matmuls cover the DMA wait so the
    HAM clock-gate reaches 2.4 GHz before real QK work; narrow warmups
    or transposes do NOT register as PE-busy for the HAM and it stays
    at 1.2 GHz for ~10us of the stream. First ACT at ~10.5us graded.
  - PV lags score production by 3 key-tiles (1 on the last head, with 2
    drains/chunk); next head's Q prep is spread over chunks 4-15.
"""

import numpy as np

SEQ = 2048
D = 128
QH = 4  # query heads per core
N_CORES = 8
SCALE = 0.08838834764831845  # 1/sqrt(128)
NT = SEQ // 128  # 16 tiles of 128 along seq

SLOT = 512              # fp32 per PSUM bank
CHUNK = 1024            # score chunk: one pool tile (2 banks), one EXP
ARENA = sum(SEQ - 128 * t for t in range(NT))  # 17408 packed score cols/head
NCH = ARENA // CHUNK    # 17 chunks per head

_NC = None


def _emit(ctx, tc, q, k, v, out):
    import concourse.mybir as mybir
    from concourse import masks

    nc = tc.nc
    f32 = mybir.dt.float32
    bf16 = mybir.dt.bfloat16
    Exp = mybir.ActivationFunctionType.Exp

    spans = [SEQ - 128 * t for t in range(NT)]
    offs = [0] * NT
    for t in range(1, NT):
        offs[t] = offs[t - 1] + spans[t - 1]

    singles = ctx.enter_context(tc.tile_pool(name="singles", bufs=1))
    qpool = ctx.enter_context(tc.tile_pool(name="qpool", bufs=2))
    apool = ctx.enter_context(tc.tile_pool(name="apool", bufs=3))
    opool = ctx.enter_context(tc.tile_pool(name="opool", bufs=3))
    # PSUM budget (8 banks): scores 2x2 + PV accumulator 2x1 + transposes 2x1.
    psum_s = ctx.enter_context(tc.tile_pool(name="psum_s", bufs=2, space="PSUM"))
    psum_o = ctx.enter_context(tc.tile_pool(name="psum_o", bufs=2, space="PSUM"))
    psum_t = ctx.enter_context(tc.tile_pool(name="psum_t", bufs=2, space="PSUM"))

    ident = singles.tile([128, 128], bf16)
    masks.make_identity(nc, ident[:])

    # PE warmup: WIDE dummy matmuls covering the DMA prologue so the HAM
    # clock-gate reaches 2.4 GHz by the time transposes + QK arrive.
    # 512-col moving operands keep the PE-busy density high enough to
    # trigger the un-throttle (128-col ones measurably do NOT).
    warm_src = singles.tile([128, 512], bf16, tag="warm_src")

    def warmup(n):
        for i in range(n):
            wsp = psum_s.tile([128, CHUNK], f32, tag="s")
            nc.tensor.matmul(
                wsp[:, 0:512], lhsT=warm_src[:, 0:128], rhs=warm_src[:],
                start=True, stop=True,
            )

    kT = singles.tile([128, SEQ], bf16)
    knat = singles.tile([128, NT, 128], f32, tag="knat")
    knat_bf = singles.tile([128, NT, 128], bf16, tag="knat_bf")
    kr = k.rearrange("(t p) d -> p t d", p=128)

    def kdma(t0, n, eng, cast=None):
        """Load + cast K tiles [t0, t0+n). Prologue loads ride the ScalarE
        hwdge queue (idle until the first ACT); steady-state loads must use
        Sync or they would delay the ACT stream. The cast engine is chosen
        per call: DVE for the latency-critical prologue chain, GpSimd for
        steady-state prefetch (so it never blocks DVE transpose copies)."""
        cs = slice(t0, t0 + n)
        eng.dma_start(out=knat[:, cs, :], in_=kr[:, cs, :])
        (cast or nc.vector).tensor_copy(knat_bf[:, cs, :], knat[:, cs, :])

    def ktp(t0, n):
        """PE-transpose K tiles [t0, t0+n) into kT."""
        for t in range(t0, t0 + n):
            pst = psum_t.tile([128, 128], bf16, tag="tp")
            nc.tensor.transpose(pst[:], knat_bf[:, t, :], ident[:])
            nc.vector.tensor_copy(kT[:, t * 128:(t + 1) * 128], pst[:])

    def qprep_alloc(h):
        qnat = singles.tile([128, NT, 128], f32, tag=f"qnat{h}")
        qnat_bf = singles.tile([128, NT, 128], bf16, tag=f"qnat_bf{h}")
        qT = qpool.tile([128, SEQ], bf16, tag="qT")
        return qnat, qnat_bf, qT

    def qprep_dma(h, st, c, cast=None):
        """Load + cast one 4-tile chunk of head h's Q. Steady-state casts
        go to GpSimd; head-0's latency-critical ones to the DVE."""
        qnat, qnat_bf, qT = st
        qrh = q[:, h * D:(h + 1) * D].rearrange("(t p) d -> p t d", p=128)
        cs = slice(c * 4, (c + 1) * 4)
        nc.sync.dma_start(out=qnat[:, cs, :], in_=qrh[:, cs, :])
        (cast or nc.vector).tensor_copy(qnat_bf[:, cs, :], qnat[:, cs, :])

    def qprep_tp(h, st, t0, n):
        """PE-transpose tiles [t0, t0+n) of head h's Q into qT."""
        qnat, qnat_bf, qT = st
        for t in range(t0, t0 + n):
            pst = psum_t.tile([128, 128], bf16, tag="tp")
            nc.tensor.transpose(pst[:], qnat_bf[:, t, :], ident[:])
            nc.vector.tensor_copy(qT[:, t * 128:(t + 1) * 128], pst[:])

    # ---- V: natural [128, t, d] bf16 + ones column for the denominator
    vp = singles.tile([128, NT, D + 1], bf16)
    vnat = singles.tile([128, NT, 128], f32, tag="vnat")

    def vprep_dma():
        nc.sync.dma_start(out=vnat[:], in_=v.rearrange("(t p) d -> p t d", p=128))
        nc.vector.memset(vp[:, :, D:D + 1], 1.0)

    def vprep_copy():
        # one wide DVE copy, scheduled AFTER the head-0 transpose copies it
        # would otherwise block on the in-order DVE queue. GpSimd is wrong
        # for this: its f32->bf16 casts run ~1.9us per 4-tile piece and
        # delay the causal masks (PV matmuls head-of-line-wait on them).
        nc.vector.tensor_copy(vp[:, :, 0:D], vnat[:])

    # Pending-PV work: a queue of sub-group entries
    # (h, qt, arena, klo, khi, final). Large qt groups (>= PV_SPLIT key
    # tiles) are split into two half-accumulations summed on the DVE, so
    # a single drain never dumps a >0.5us matmul burst on the PE (whole
    # big groups stall the ACT stream at head transitions otherwise).
    PV_SPLIT = 13
    pvq = []
    pv_state = {}

    def pv_queue(h2, qt2, arena2):
        if qt2 >= PV_SPLIT:
            half = (qt2 + 1) // 2
            pvq.append((h2, qt2, arena2, 0, half, False))
            pvq.append((h2, qt2, arena2, half, qt2 + 1, True))
        else:
            pvq.append((h2, qt2, arena2, 0, qt2 + 1, True))

    def pv_advance(budget_mms, max_groups=2):
        """Pop sub-group entries (each an atomic accumulation run) until
        the MM budget is spent. O[qt] = sum_k2 arena[k2-slice].T @ [V|1];
        the ones column gives the softmax denominator for free."""
        left = budget_mms
        groups = 0
        while left > 0 and groups < max_groups and pvq:
            groups += 1
            h2, qt2, arena2, klo, khi, final = pvq.pop(0)
            st = pv_state.setdefault(h2, {})
            ops = psum_o.tile([128, D + 1], f32, tag="o")
            if qt2 % 2 == 0 and klo == 0:
                st["osb"] = opool.tile([128, 2, D], f32, tag="osb", name="osb")
            for kk in range(klo, khi):
                a0 = offs[kk] + (qt2 - kk) * 128
                nc.tensor.matmul(
                    ops[:], lhsT=arena2[:, a0:a0 + 128], rhs=vp[:, kk, :],
                    start=(kk == klo), stop=(kk == khi - 1),
                )
            left -= khi - klo
            if not final:
                # first half: bounce to SBUF so the PSUM bank frees fast
                half_sb = opool.tile([128, D + 1], f32, tag="half", name="half_sb")
                nc.vector.tensor_copy(half_sb[:], ops[:])
                st[("half", qt2)] = half_sb
                continue
            if klo > 0:
                half_sb = st.pop(("half", qt2))
                tot = opool.tile([128, D + 1], f32, tag="tot")
                nc.vector.scalar_tensor_tensor(
                    tot[:], ops[:], 1.0, half_sb[:],
                    mybir.AluOpType.mult, mybir.AluOpType.add,
                )
                src = tot
            else:
                src = ops
            osb = st["osb"]
            rec = opool.tile([128, 1], f32, tag="rec")
            nc.vector.reciprocal(rec[:], src[:, D:D + 1])
            nc.vector.tensor_scalar_mul(osb[:, qt2 % 2, :], src[:, 0:D], rec[:])
            if h2 == QH - 1 and qt2 >= 12:
                # overlap the last stores with the trailing normalizations
                nc.sync.dma_start(
                    out=out[qt2 * 128:(qt2 + 1) * 128, h2 * D:(h2 + 1) * D],
                    in_=osb[:, qt2 % 2, :],
                )
            elif qt2 % 2 == 1:
                qb = qt2 // 2
                nc.sync.dma_start(
                    out=out[qb * 256:(qb + 1) * 256, h2 * D:(h2 + 1) * D].rearrange(
                        "(j p) d -> p j d", p=128
                    ),
                    in_=osb[:],
                )

    # Prologue, critical-bytes first: chunk 0's QK needs ONLY K tile 0
    # (64KB) + Q tiles 0-7; chunk 1 adds Q tiles 8-15. In-flight DMAs
    # fair-share the HBM fabric, so everything else (K tiles 1-15, V) is
    # deferred into the chunk loop rather than issued up front — issuing
    # it all at once starves the critical transfers. K rides the ScalarE
    # hwdge queue (idle until the first ACT), Q rides Sync.
    nc.vector.memset(warm_src[:], 0.0)
    kdma(0, 1, nc.scalar, cast=nc.vector)
    q0st = qprep_alloc(0)
    qprep_dma(0, q0st, 0, cast=nc.vector)
    qprep_dma(0, q0st, 1, cast=nc.vector)
    warmup(8)
    ktp(0, 1)
    qprep_tp(0, q0st, 0, 4)
    qprep_tp(0, q0st, 4, 4)
    # c2/c3 casts are emitted AFTER the tile 0-7 transpose copies so they
    # never block them on the in-order DVE queue; tiles 8-11 transpose in
    # the PE's idle window before chunk 0's QK.
    qprep_dma(0, q0st, 2, cast=nc.vector)
    qprep_dma(0, q0st, 3, cast=nc.vector)
    qprep_tp(0, q0st, 8, 4)
    qT = q0st[2]

    for h in range(QH):
        arena = apool.tile([128, ARENA], bf16, tag="arena")
        qT_next = None
        qst_next = None
        done_kt = 0       # key-tiles fully exp'd so far
        done_mask = 0     # key-tiles whose diagonal block is masked
        for ci in range(NCH):
            c0, c1 = ci * CHUNK, (ci + 1) * CHUNK
            # late prologue interleave (head 0 only): K DMAs go early so the
            # casts/transposes never stall the PE mid-head.
            if h == 0:
                if ci == 0:
                    kdma(1, 3, nc.scalar, cast=nc.vector)
                    vprep_dma()
                elif ci == 1:
                    # chunk 1 reads qT cols 1024-2048: tiles 12-15 must be
                    # transposed before this chunk's QK matmuls.
                    qprep_tp(0, q0st, 12, 4)
                elif ci == 2:
                    # kT tiles 1-2 are first read by this chunk's / chunk
                    # 3's QK, and the transposes emit ahead of them.
                    ktp(1, 2)
                    vprep_copy()
                elif ci == 3:
                    kdma(4, 4, nc.sync, cast=nc.vector)
                elif ci == 4:
                    ktp(3, 1)
                elif ci == 5:
                    kdma(8, 4, nc.sync, cast=nc.vector)
                    ktp(4, 4)
                elif ci == 9:
                    kdma(12, 4, nc.sync, cast=nc.vector)
                    ktp(8, 4)
                elif ci == 12:
                    ktp(12, 4)
            sp = psum_s.tile([128, CHUNK], f32, tag="s")
            # QK matmul fragments: split at PSUM bank boundaries
            for kt in range(NT):
                s0, s1 = max(c0, offs[kt]), min(c1, offs[kt] + spans[kt])
                if s0 >= s1:
                    continue
                p = s0
                while p < s1:
                    w = min(s1 - p, SLOT - (p % SLOT))
                    qs = kt * 128 + (p - offs[kt])
                    nc.tensor.matmul(
                        sp[:, p - c0:p - c0 + w],
                        lhsT=kT[:, kt * 128:(kt + 1) * 128],
                        rhs=qT[:, qs:qs + w],
                        start=True, stop=True,
                    )
                    p += w
            # one wide exp over the whole chunk
            nc.scalar.activation(arena[:, c0:c1], sp[:], Exp, scale=SCALE)
            # mask diagonal blocks in place (GpSimd, idle queue) as soon as
            # their 128 cols are exp'd, and queue PV on full completion.
            while done_mask < NT and offs[done_mask] + 128 <= c1:
                o0 = offs[done_mask]
                nc.gpsimd.affine_select(
                    out=arena[:, o0:o0 + 128], in_=arena[:, o0:o0 + 128],
                    compare_op=mybir.AluOpType.is_ge, fill=0.0,
                    base=0, channel_multiplier=-1, pattern=[[1, 128]],
                )
                done_mask += 1
            while done_kt < NT and offs[done_kt] + spans[done_kt] <= c1:
                pv_queue(h, done_kt, arena)
                done_kt += 1
            # lag-based PV drain
            lag = 1 if h == QH - 1 else 3
            drains = 2 if h == QH - 1 else 1
            for _ in range(drains):
                if len(pvq) > lag:
                    pv_advance(17, max_groups=1)
            # spread the next head's Q prep over late chunks: DMAs at
            # ci 4/6/8/10 (two chunks of slack before their transposes
            # need the casts), two transposes per chunk at ci 8..15.
            if h + 1 < QH:
                if ci == 4:
                    qst_next = qprep_alloc(h + 1)
                    qT_next = qst_next[2]
                if ci in (4, 6, 8, 10):
                    qprep_dma(h + 1, qst_next, (ci - 4) // 2)
                if 8 <= ci <= 15:
                    qprep_tp(h + 1, qst_next, (ci - 8) * 2, 2)
        if qT_next is not None:
            qT = qT_next
    while pvq:
        pv_advance(17, max_groups=1)


def _build():
    import concourse.mybir as mybir
    import concourse.tile as tile
    from concourse import bacc
    from contextlib import ExitStack

    nc = bacc.Bacc()
    q = nc.declare_dram_parameter("q", [SEQ, QH * D], mybir.dt.float32, isOutput=False)
    k = nc.declare_dram_parameter("k", [SEQ, D], mybir.dt.float32, isOutput=False)
    v = nc.declare_dram_parameter("v", [SEQ, D], mybir.dt.float32, isOutput=False)
    out = nc.declare_dram_parameter("out", [SEQ, QH * D], mybir.dt.float32, isOutput=True)

    with tile.TileContext(nc) as tc:
        with ExitStack() as ctx:
            _emit(ctx, tc, q[:], k[:], v[:], out[:])
    nc.compile()
    return nc


def _get_nc():
    global _NC
    if _NC is None:
        _NC = _build()
    return _NC


def _ensure_ntff_hook():
    """The agent image's antenv lacks axon_hooks; shim it so trace=True works."""
    import sys
    import types

    if "antenv.axon_hooks" in sys.modules:
        return
    try:
        import antenv
        from trn_agent_boot.trn_boot import _ntff_profile_via_ctypes
    except ImportError:
        return
    mod = types.ModuleType("antenv.axon_hooks")
    hook = [None]
    mod.set_axon_ntff_profile_hook = lambda h: hook.__setitem__(0, h)
    mod.get_axon_ntff_profile_hook = lambda: hook[0]
    sys.modules["antenv.axon_hooks"] = mod
    antenv.axon_hooks = mod
    mod.set_axon_ntff_profile_hook(_ntff_profile_via_ctypes("/opt/axon/libaxon_pjrt.so"))


def _run(q, k, v, trace=False):
    from concourse.bass_utils import run_bass_kernel_spmd

    if trace:
        _ensure_ntff_hook()
    nc = _get_nc()
    in_maps = []
    for i in range(N_CORES):
        in_maps.append(
            {
                "q": np.ascontiguousarray(q[:, i * QH * D:(i + 1) * QH * D]).astype(np.float32, copy=False),
                "k": np.ascontiguousarray(k[:, i * D:(i + 1) * D]).astype(np.float32, copy=False),
                "v": np.ascontiguousarray(v[:, i * D:(i + 1) * D]).astype(np.float32, copy=False),
            }
        )
    res = run_bass_kernel_spmd(nc, in_maps, core_ids=list(range(N_CORES)), trace=trace)
    full = np.concatenate([res.results[i]["out"] for i in range(N_CORES)], axis=1)
    return full.astype(np.float32, copy=False), res


def kernel(q, k, v):
    out, _ = _run(q, k, v, trace=False)
    return out
